# revision 4
# baseline (speedup 1.0000x reference)
"""Trainium2 Bass kernel for BasicGNN message passing.

out = x @ W_s + scatter_add(norm_e * (x @ W_n)[col_e] -> row_e) + bias

Key algebraic restructure: aggregate x first, transform after:
    agg[r] = sum_{e: row_e=r} norm_e * x[col_e]          (sparse, memory-bound)
    out    = x @ W_s + agg @ W_n + bias                  (dense, tensor engine)
This avoids materializing h = x @ W_n for all nodes entirely.

Sharding: nodes (output rows) split contiguously across 8 cores; edges
partitioned by destination row. Each core gathers source rows of x directly
from its own full copy of x in DRAM (no collectives).

Device algorithm per core:
  - edges sorted by dst window (128 rows) and source-quarter (dma_gather has
    int16 indices, so x is addressed in 4 quarters of 25000 rows)
  - dma_gather fetches x[col] rows, 128 per chunk (one edge per partition)
  - scalar engine scales each gathered row by norm_e
  - vector engine builds a one-hot selection matrix S[e, d] = (dst_local[e]==d)
  - tensor engine: psum[d, f] += S.T @ msg  accumulates the scatter-add
  - per window: transpose agg tile to [f, d] via tensor engine
  - final: out_T = W_s.T @ x_T + W_n.T @ agg_T + bias, streamed per 512 rows
"""

import sys

if "/opt/trn_rl_repo" not in sys.path:
    sys.path.insert(0, "/opt/trn_rl_repo")

import numpy as np

import concourse.bass as bass
import concourse.mybir as mybir
import concourse.tile as tile
from concourse import bacc
from concourse.bass_utils import run_bass_kernel_spmd
from concourse.masks import make_identity

N_NODES = 100000
N_EDGES = 1600000
D = 128
NC = 8
ROWS_PER_CORE = N_NODES // NC          # 12500
WIN = 128                              # dst rows per psum window
W = (ROWS_PER_CORE + WIN - 1) // WIN   # 98 windows per core
NQ = 4                                 # source quarters (int16 index limit)
QSIZE = N_NODES // NQ                  # 25000
GROUP = 4                              # windows per gather call group
F32 = mybir.dt.float32
I16 = mybir.dt.int16


def _build_schedule(counts):
    """counts: [NC, W, NQ] edge counts. Returns shared schedule.

    Schedule entries (all cores share shapes/loop structure):
      n_chunks[w][q] = max over cores of ceil(count/128)   (0 if all zero)
      groups: list of (list of windows)
      calls: per group, per q: (tok_base, n_chunk_call)
      window_chunks[w]: list of global chunk column indices (dstl/norm cols)
    """
    n_chunks = np.ceil(counts / 128.0).astype(np.int64).max(axis=0)  # [W, NQ]
    groups = [list(range(g, min(g + GROUP, W))) for g in range(0, W, GROUP)]
    calls = []            # (g, q, tok_base, call_chunks, chunk_base)
    window_chunks = {w: [] for w in range(W)}
    tok = 0
    chunk = 0
    for gi, wins in enumerate(groups):
        for q in range(NQ):
            call_chunks = int(sum(n_chunks[w][q] for w in wins))
            if call_chunks == 0:
                continue
            calls.append((gi, q, tok, call_chunks, chunk))
            for w in wins:
                for _ in range(int(n_chunks[w][q])):
                    window_chunks[w].append(chunk)
                    chunk += 1
            tok += call_chunks * 128
    return n_chunks, groups, calls, window_chunks, tok, chunk


def _prep_core(core, row_s, col_s, norm_s, bounds, n_chunks, groups, total_tok,
               total_chunk):
    """Build per-core token arrays (idx int16 wrapped, dstl, norm)."""
    lo, hi = bounds[core], bounds[core + 1]
    r = row_s[lo:hi] - core * ROWS_PER_CORE
    c = col_s[lo:hi]
    nm = norm_s[lo:hi]
    warr = r // WIN
    qarr = c // QSIZE
    order = np.lexsort((qarr, warr))
    r, c, nm, warr, qarr = r[order], c[order], nm[order], warr[order], qarr[order]
    dstl = (r % WIN).astype(np.float32)
    cloc = (c % QSIZE).astype(np.int16)

    # segment starts per (w, q) via searchsorted on combined key
    key = warr * NQ + qarr
    idx_tok = np.zeros(total_tok, dtype=np.int16)
    dstl_tok = np.zeros(total_tok, dtype=np.float32)
    norm_tok = np.zeros(total_tok, dtype=np.float32)
    pos = 0
    for wins in groups:
        for q in range(NQ):
            for w in wins:
                k = int(n_chunks[w][q])
                if k == 0:
                    continue
                s = np.searchsorted(key, w * NQ + q, side="left")
                e = np.searchsorted(key, w * NQ + q, side="right")
                m = e - s
                assert m <= k * 128
                idx_tok[pos:pos + m] = cloc[s:e]
                dstl_tok[pos:pos + m] = dstl[s:e]
                norm_tok[pos:pos + m] = nm[s:e]
                pos += k * 128
    assert pos == total_tok

    idx_wrap = idx_tok.reshape(total_tok // 16, 16).T.copy()       # [16, T/16]
    idx_rep = np.tile(idx_wrap, (8, 1))                            # [128, T/16]
    dstl_m = dstl_tok.reshape(total_chunk, 128).T.copy()           # [128, C]
    norm_m = norm_tok.reshape(total_chunk, 128).T.copy()           # [128, C]
    return idx_rep, dstl_m, norm_m


def _build_program(n_chunks, groups, calls, window_chunks, total_tok,
                   total_chunk):
    nc = bacc.Bacc("TRN2", target_bir_lowering=False, debug=False,
                   num_devices=NC)
    x_d = nc.dram_tensor("x", [N_NODES, D], F32, kind="ExternalInput").ap()
    xT_d = nc.dram_tensor("xT", [D, ROWS_PER_CORE], F32,
                          kind="ExternalInput").ap()
    idx_d = nc.dram_tensor("idx", [128, total_tok // 16], I16,
                           kind="ExternalInput").ap()
    dstl_d = nc.dram_tensor("dstl", [128, total_chunk], F32,
                            kind="ExternalInput").ap()
    norm_d = nc.dram_tensor("norm", [128, total_chunk], F32,
                            kind="ExternalInput").ap()
    ws_d = nc.dram_tensor("ws", [D, D], F32, kind="ExternalInput").ap()
    wn_d = nc.dram_tensor("wn", [D, D], F32, kind="ExternalInput").ap()
    bias_d = nc.dram_tensor("bias", [D, 1], F32, kind="ExternalInput").ap()
    iota_d = nc.dram_tensor("iota", [128, 128], F32, kind="ExternalInput").ap()
    outT_d = nc.dram_tensor("outT", [D, ROWS_PER_CORE], F32,
                            kind="ExternalOutput").ap()

    # group -> token extents for idx slab loads
    call_by_group = {}
    for gi, q, tok, cc, cb in calls:
        call_by_group.setdefault(gi, []).append((q, tok, cc, cb))
    group_tok = {gi: (cl[0][1], cl[-1][1] + cl[-1][2] * 128)
                 for gi, cl in call_by_group.items()}

    with tile.TileContext(nc) as tc:
        with (
            tc.tile_pool(name="const", bufs=1) as constp,
            tc.tile_pool(name="meta", bufs=1) as metap,
            tc.tile_pool(name="aggTp", bufs=1) as aggTp,
            tc.tile_pool(name="idxg", bufs=3) as idxgp,
            tc.tile_pool(name="xg", bufs=8) as xgp,
            tc.tile_pool(name="S", bufs=4) as Sp,
            tc.tile_pool(name="msg", bufs=4) as msgp,
            tc.tile_pool(name="aggsb", bufs=3) as aggsbp,
            tc.tile_pool(name="xT", bufs=2) as xTp,
            tc.tile_pool(name="osb", bufs=2) as osbp,
            tc.tile_pool(name="psA", bufs=2, space="PSUM") as psA,
            tc.tile_pool(name="psT", bufs=2, space="PSUM") as psT,
            tc.tile_pool(name="psO", bufs=2, space="PSUM") as psO,
        ):
            ident = constp.tile([128, 128], F32)
            make_identity(nc, ident[:])
            iota_t = constp.tile([128, 128], F32)
            nc.sync.dma_start(iota_t[:], iota_d[:])
            ws_t = constp.tile([D, D], F32)
            nc.sync.dma_start(ws_t[:], ws_d[:])
            wn_t = constp.tile([D, D], F32)
            nc.sync.dma_start(wn_t[:], wn_d[:])
            bias_t = constp.tile([D, 1], F32)
            nc.sync.dma_start(bias_t[:], bias_d[:])
            dstl_t = metap.tile([128, total_chunk], F32)
            nc.sync.dma_start(dstl_t[:], dstl_d[:])
            norm_t = metap.tile([128, total_chunk], F32)
            nc.sync.dma_start(norm_t[:], norm_d[:])
            aggT = aggTp.tile([128, W * WIN], F32)

            for gi, wins in enumerate(groups):
                t0, t1 = group_tok[gi]
                idx_t = idxgp.tile([128, (t1 - t0) // 16], I16, tag="idxg")
                nc.sync.dma_start(idx_t[:], idx_d[:, t0 // 16:t1 // 16])
                # gather calls for this group's quarters
                bufs = {}
                for q, tok, cc, cb in call_by_group[gi]:
                    xg = xgp.tile([128, cc * 128], F32, tag="xg")
                    nc.gpsimd.dma_gather(
                        out_ap=xg[:].rearrange("p (c e) -> p c e", e=128),
                        in_ap=x_d[q * QSIZE:(q + 1) * QSIZE, :],
                        idxs_ap=idx_t[:, (tok - t0) // 16:
                                      (tok - t0 + cc * 128) // 16],
                        num_idxs=cc * 128,
                        num_idxs_reg=cc * 128,
                        elem_size=D,
                        queue_num=0,
                        single_packet=False,
                    )
                    bufs[q] = (xg, cb)
                # scatter matmuls per window
                for w in wins:
                    chunks = window_chunks[w]
                    psum_agg = psA.tile([128, 128], F32, tag="psA")
                    if not chunks:
                        agg_sb = aggsbp.tile([128, 128], F32, tag="aggsb")
                        nc.vector.memset(agg_sb[:], 0.0)
                    else:
                        for ci, gc in enumerate(chunks):
                            # locate owning call buffer
                            q = None
                            for qq, (xg, cb) in bufs.items():
                                nchq = sum(int(n_chunks[ww][qq]) for ww in wins)
                                if cb <= gc < cb + nchq:
                                    q = qq
                                    break
                            xg, cb = bufs[q]
                            j = gc - cb
                            S = Sp.tile([128, 128], F32, tag="S")
                            nc.vector.tensor_tensor(
                                out=S[:],
                                in0=dstl_t[:, gc:gc + 1].to_broadcast([128, 128]),
                                in1=iota_t[:],
                                op=mybir.AluOpType.is_equal,
                            )
                            msg = msgp.tile([128, 128], F32, tag="msg")
                            nc.scalar.activation(
                                out=msg[:],
                                in_=xg[:, j * 128:(j + 1) * 128],
                                func=mybir.ActivationFunctionType.Copy,
                                scale=norm_t[:, gc:gc + 1],
                            )
                            nc.tensor.matmul(
                                out=psum_agg[:],
                                lhsT=S[:],
                                rhs=msg[:],
                                start=(ci == 0),
                                stop=(ci == len(chunks) - 1),
                            )
                        agg_sb = aggsbp.tile([128, 128], F32, tag="aggsb")
                        nc.vector.tensor_copy(out=agg_sb[:], in_=psum_agg[:])
                    psum_t = psT.tile([128, 128], F32, tag="psT")
                    nc.tensor.transpose(psum_t[:], agg_sb[:], ident[:])
                    nc.vector.tensor_copy(
                        out=aggT[:, w * WIN:(w + 1) * WIN], in_=psum_t[:])

            # final dense phase: out_T = W_s.T @ x_T + W_n.T @ agg_T + bias
            TS = 512
            for t in range(0, ROWS_PER_CORE, TS):
                n = min(TS, ROWS_PER_CORE - t)
                xT_t = xTp.tile([128, n], F32, tag="xT")
                nc.sync.dma_start(xT_t[:], xT_d[:, t:t + n])
                psum_o = psO.tile([128, n], F32, tag="psO")
                nc.tensor.matmul(out=psum_o[:], lhsT=ws_t[:], rhs=xT_t[:],
                                 start=True, stop=False)
                nc.tensor.matmul(out=psum_o[:], lhsT=wn_t[:],
                                 rhs=aggT[:, t:t + n], start=False, stop=True)
                osb = osbp.tile([128, n], F32, tag="osb")
                nc.vector.tensor_scalar_add(osb[:], psum_o[:], bias_t[:, :1])
                nc.sync.dma_start(outT_d[:, t:t + n], osb[:])
    nc.compile()
    return nc


def kernel(x, edge_index, self_weight, neighbor_weight, bias):
    x = np.asarray(x, dtype=np.float32)
    edge_index = np.asarray(edge_index)
    self_weight = np.asarray(self_weight, dtype=np.float32)
    neighbor_weight = np.asarray(neighbor_weight, dtype=np.float32)
    bias = np.asarray(bias, dtype=np.float32)

    row = edge_index[0].astype(np.int64)
    col = edge_index[1].astype(np.int64)

    deg = np.bincount(row, minlength=N_NODES).astype(np.float32)
    with np.errstate(divide="ignore"):
        dis = deg ** -0.5
    norm = (dis[row] * dis[col]).astype(np.float32)

    order = np.argsort(row, kind="stable")
    row_s, col_s, norm_s = row[order], col[order], norm[order]
    bounds = np.searchsorted(row_s, np.arange(NC + 1) * ROWS_PER_CORE)

    # per (core, window, quarter) counts for the shared schedule
    counts = np.zeros((NC, W, NQ), dtype=np.int64)
    wid = (row_s % ROWS_PER_CORE) // WIN
    qid = col_s // QSIZE
    cid = row_s // ROWS_PER_CORE
    np.add.at(counts, (cid, wid, qid), 1)

    n_chunks, groups, calls, window_chunks, total_tok, total_chunk = (
        _build_schedule(counts))

    nc = _build_program(n_chunks, groups, calls, window_chunks, total_tok,
                        total_chunk)

    iota = np.tile(np.arange(128, dtype=np.float32), (128, 1))
    in_maps = []
    for c in range(NC):
        idx_rep, dstl_m, norm_m = _prep_core(
            c, row_s, col_s, norm_s, bounds, n_chunks, groups, total_tok,
            total_chunk)
        in_maps.append({
            "x": x,
            "xT": np.ascontiguousarray(
                x[c * ROWS_PER_CORE:(c + 1) * ROWS_PER_CORE].T),
            "idx": idx_rep,
            "dstl": dstl_m,
            "norm": norm_m,
            "ws": self_weight,
            "wn": neighbor_weight,
            "bias": bias.reshape(D, 1),
            "iota": iota,
        })

    global _LAST
    _LAST = (nc, in_maps)
    res = run_bass_kernel_spmd(nc, in_maps, list(range(NC)))
    out = np.empty((N_NODES, D), dtype=np.float32)
    for c in range(NC):
        out[c * ROWS_PER_CORE:(c + 1) * ROWS_PER_CORE] = res.results[c]["outT"].T
    return out


_LAST = None


def profile_exec_ns():
    """Re-run the last-built program with NTFF tracing; returns exec ns."""
    assert _LAST is not None, "call kernel() first"
    nc, in_maps = _LAST
    res = run_bass_kernel_spmd(nc, in_maps, list(range(NC)), trace=True)
    return res.exec_time_ns


# revision 5
# speedup vs baseline: 1.4937x; 1.4937x over previous
"""Trainium2 Bass kernel for BasicGNN message passing.

out = x @ W_s + scatter_add(norm_e * (x @ W_n)[col_e] -> row_e) + bias

Algebraic restructures:
  1. Aggregate x first, transform after (avoids materializing h = x @ W_n):
         agg[r] = sum_{e: row_e=r} norm_e * x[col_e]
         out    = x @ W_s + agg @ W_n + bias
  2. norm is separable: norm_e = dis[row_e] * dis[col_e] with
     dis = deg^-1/2. The dis[col] factor is pre-multiplied into the gather
     table on the host (x' = dis * x); the dis[row] factor is applied once
     per 128-row destination window when copying PSUM -> SBUF.

Sharding: output rows split contiguously across 8 cores; edges partitioned
by destination row. Each core gathers source rows of x' from its own full
copy in DRAM (no collectives).

Device algorithm per core:
  - edges sorted by dst window (128 rows) and source-quarter (dma_gather
    indices are int16, so the table is addressed in 4 quarters of 25001
    rows - the extra row per quarter is a zero sentinel for padding)
  - dma_gather (4 SWDGE queues round-robin) fetches x'[col] rows, 128 per
    chunk (one edge per partition)
  - vector engine builds one-hot S[e, d] = (dst_local[e]==d)
  - tensor engine: psum[d, f] += S.T @ xg accumulates the scatter-add
  - scalar engine applies dis[row] while copying psum -> SBUF
  - per window: transpose agg tile to [f, d] via tensor engine
  - final: out_T = W_s.T @ x_T + W_n.T @ agg_T + bias, streamed per 512 rows
"""

import sys

if "/opt/trn_rl_repo" not in sys.path:
    sys.path.insert(0, "/opt/trn_rl_repo")

import numpy as np

import concourse.bass as bass
import concourse.mybir as mybir
import concourse.tile as tile
from concourse import bacc
from concourse.bass_utils import run_bass_kernel_spmd
from concourse.masks import make_identity

N_NODES = 100000
N_EDGES = 1600000
D = 128
NC = 8
ROWS_PER_CORE = N_NODES // NC          # 12500
WIN = 128                              # dst rows per psum window
W = (ROWS_PER_CORE + WIN - 1) // WIN   # 98 windows per core
NQ = 4                                 # source quarters (int16 index limit)
QSIZE = N_NODES // NQ                  # 25000
QROWS = QSIZE + 1                      # + zero sentinel row per quarter
GROUP = 4                              # windows per gather call group
F32 = mybir.dt.float32
I16 = mybir.dt.int16


def _build_schedule(counts):
    """counts: [NC, W, NQ] edge counts. Returns shared schedule (all cores
    share shapes / loop structure; per-(w,q) chunk counts are max over
    cores)."""
    n_chunks = np.ceil(counts / 128.0).astype(np.int64).max(axis=0)  # [W, NQ]
    groups = [list(range(g, min(g + GROUP, W))) for g in range(0, W, GROUP)]
    calls = []            # (g, q, tok_base, call_chunks, chunk_base)
    window_chunks = {w: [] for w in range(W)}
    tok = 0
    chunk = 0
    for gi, wins in enumerate(groups):
        for q in range(NQ):
            call_chunks = int(sum(n_chunks[w][q] for w in wins))
            if call_chunks == 0:
                continue
            calls.append((gi, q, tok, call_chunks, chunk))
            for w in wins:
                for _ in range(int(n_chunks[w][q])):
                    window_chunks[w].append(chunk)
                    chunk += 1
            tok += call_chunks * 128
    return n_chunks, groups, calls, window_chunks, tok, chunk


def _prep_core(core, row_s, col_s, bounds, n_chunks, groups, total_tok,
               total_chunk):
    """Per-core token arrays: idx (int16 wrapped+replicated) and dstl."""
    lo, hi = bounds[core], bounds[core + 1]
    r = row_s[lo:hi] - core * ROWS_PER_CORE
    c = col_s[lo:hi]
    warr = r // WIN
    qarr = c // QSIZE
    order = np.lexsort((qarr, warr))
    r, c, warr, qarr = r[order], c[order], warr[order], qarr[order]
    dstl = (r % WIN).astype(np.float32)
    cloc = (c % QSIZE).astype(np.int16)

    key = warr * NQ + qarr
    idx_tok = np.full(total_tok, QSIZE, dtype=np.int16)   # sentinel pad
    dstl_tok = np.zeros(total_tok, dtype=np.float32)
    pos = 0
    for wins in groups:
        for q in range(NQ):
            for w in wins:
                k = int(n_chunks[w][q])
                if k == 0:
                    continue
                s = np.searchsorted(key, w * NQ + q, side="left")
                e = np.searchsorted(key, w * NQ + q, side="right")
                m = e - s
                assert m <= k * 128
                idx_tok[pos:pos + m] = cloc[s:e]
                dstl_tok[pos:pos + m] = dstl[s:e]
                pos += k * 128
    assert pos == total_tok

    idx_wrap = idx_tok.reshape(total_tok // 16, 16).T.copy()       # [16, T/16]
    idx_rep = np.tile(idx_wrap, (8, 1))                            # [128, T/16]
    dstl_m = dstl_tok.reshape(total_chunk, 128).T.copy()           # [128, C]
    return idx_rep, dstl_m


def _build_program(n_chunks, groups, calls, window_chunks, total_tok,
                   total_chunk):
    nc = bacc.Bacc("TRN2", target_bir_lowering=False, debug=False,
                   num_devices=NC, num_swdge_queues=4)
    xs_d = nc.dram_tensor("xs", [NQ * QROWS, D], F32,
                          kind="ExternalInput").ap()
    xT_d = nc.dram_tensor("xT", [D, ROWS_PER_CORE], F32,
                          kind="ExternalInput").ap()
    idx_d = nc.dram_tensor("idx", [128, total_tok // 16], I16,
                           kind="ExternalInput").ap()
    dstl_d = nc.dram_tensor("dstl", [128, total_chunk], F32,
                            kind="ExternalInput").ap()
    disw_d = nc.dram_tensor("disw", [128, W], F32, kind="ExternalInput").ap()
    ws_d = nc.dram_tensor("ws", [D, D], F32, kind="ExternalInput").ap()
    wn_d = nc.dram_tensor("wn", [D, D], F32, kind="ExternalInput").ap()
    bias_d = nc.dram_tensor("bias", [D, 1], F32, kind="ExternalInput").ap()
    iota_d = nc.dram_tensor("iota", [128, 128], F32, kind="ExternalInput").ap()
    outT_d = nc.dram_tensor("outT", [D, ROWS_PER_CORE], F32,
                            kind="ExternalOutput").ap()

    call_by_group = {}
    for gi, q, tok, cc, cb in calls:
        call_by_group.setdefault(gi, []).append((q, tok, cc, cb))
    group_tok = {gi: (cl[0][1], cl[-1][1] + cl[-1][2] * 128)
                 for gi, cl in call_by_group.items()}

    qn = 0  # SWDGE queue round-robin counter

    with tile.TileContext(nc) as tc:
        with (
            tc.tile_pool(name="const", bufs=1) as constp,
            tc.tile_pool(name="meta", bufs=1) as metap,
            tc.tile_pool(name="aggTp", bufs=1) as aggTp,
            tc.tile_pool(name="idxg", bufs=3) as idxgp,
            tc.tile_pool(name="xg", bufs=8) as xgp,
            tc.tile_pool(name="S", bufs=6) as Sp,
            tc.tile_pool(name="aggsb", bufs=3) as aggsbp,
            tc.tile_pool(name="xT", bufs=2) as xTp,
            tc.tile_pool(name="osb", bufs=2) as osbp,
            tc.tile_pool(name="psA", bufs=2, space="PSUM") as psA,
            tc.tile_pool(name="psT", bufs=2, space="PSUM") as psT,
            tc.tile_pool(name="psO", bufs=2, space="PSUM") as psO,
        ):
            ident = constp.tile([128, 128], F32)
            make_identity(nc, ident[:])
            iota_t = constp.tile([128, 128], F32)
            nc.sync.dma_start(iota_t[:], iota_d[:])
            ws_t = constp.tile([D, D], F32)
            nc.sync.dma_start(ws_t[:], ws_d[:])
            wn_t = constp.tile([D, D], F32)
            nc.sync.dma_start(wn_t[:], wn_d[:])
            bias_t = constp.tile([D, 1], F32)
            nc.sync.dma_start(bias_t[:], bias_d[:])
            disw_t = constp.tile([128, W], F32)
            nc.sync.dma_start(disw_t[:], disw_d[:])
            dstl_t = metap.tile([128, total_chunk], F32)
            nc.sync.dma_start(dstl_t[:], dstl_d[:])
            aggT = aggTp.tile([128, W * WIN], F32)

            for gi, wins in enumerate(groups):
                t0, t1 = group_tok[gi]
                idx_t = idxgp.tile([128, (t1 - t0) // 16], I16, tag="idxg")
                nc.sync.dma_start(idx_t[:], idx_d[:, t0 // 16:t1 // 16])
                bufs = {}
                for q, tok, cc, cb in call_by_group[gi]:
                    xg = xgp.tile([128, cc * 128], F32, tag="xg")
                    nc.gpsimd.dma_gather(
                        out_ap=xg[:].rearrange("p (c e) -> p c e", e=128),
                        in_ap=xs_d[q * QROWS:(q + 1) * QROWS, :],
                        idxs_ap=idx_t[:, (tok - t0) // 16:
                                      (tok - t0 + cc * 128) // 16],
                        num_idxs=cc * 128,
                        num_idxs_reg=cc * 128,
                        elem_size=D,
                        queue_num=qn % 4,
                        single_packet=False,
                    )
                    qn += 1
                    bufs[q] = (xg, cb)
                for w in wins:
                    chunks = window_chunks[w]
                    psum_agg = psA.tile([128, 128], F32, tag="psA")
                    agg_sb = aggsbp.tile([128, 128], F32, tag="aggsb")
                    if not chunks:
                        nc.vector.memset(agg_sb[:], 0.0)
                    else:
                        for ci, gc in enumerate(chunks):
                            q = None
                            for qq, (xgb, cb) in bufs.items():
                                nchq = sum(int(n_chunks[ww][qq])
                                           for ww in wins)
                                if cb <= gc < cb + nchq:
                                    q = qq
                                    break
                            xgb, cb = bufs[q]
                            j = gc - cb
                            S = Sp.tile([128, 128], F32, tag="S")
                            nc.vector.tensor_tensor(
                                out=S[:],
                                in0=dstl_t[:, gc:gc + 1].to_broadcast(
                                    [128, 128]),
                                in1=iota_t[:],
                                op=mybir.AluOpType.is_equal,
                            )
                            nc.tensor.matmul(
                                out=psum_agg[:],
                                lhsT=S[:],
                                rhs=xgb[:, j * 128:(j + 1) * 128],
                                start=(ci == 0),
                                stop=(ci == len(chunks) - 1),
                            )
                        # psum -> sbuf with the dis[row] factor applied
                        nc.scalar.activation(
                            out=agg_sb[:], in_=psum_agg[:],
                            func=mybir.ActivationFunctionType.Copy,
                            scale=disw_t[:, w:w + 1])
                    psum_t = psT.tile([128, 128], F32, tag="psT")
                    nc.tensor.transpose(psum_t[:], agg_sb[:], ident[:])
                    nc.vector.tensor_copy(
                        out=aggT[:, w * WIN:(w + 1) * WIN], in_=psum_t[:])

            # final dense phase: out_T = W_s.T @ x_T + W_n.T @ agg_T + bias
            TS = 512
            for t in range(0, ROWS_PER_CORE, TS):
                n = min(TS, ROWS_PER_CORE - t)
                xT_t = xTp.tile([128, n], F32, tag="xT")
                nc.sync.dma_start(xT_t[:], xT_d[:, t:t + n])
                psum_o = psO.tile([128, n], F32, tag="psO")
                nc.tensor.matmul(out=psum_o[:], lhsT=ws_t[:], rhs=xT_t[:],
                                 start=True, stop=False)
                nc.tensor.matmul(out=psum_o[:], lhsT=wn_t[:],
                                 rhs=aggT[:, t:t + n], start=False, stop=True)
                osb = osbp.tile([128, n], F32, tag="osb")
                nc.vector.tensor_scalar_add(osb[:], psum_o[:], bias_t[:, :1])
                nc.sync.dma_start(outT_d[:, t:t + n], osb[:])
    nc.compile()
    return nc


def kernel(x, edge_index, self_weight, neighbor_weight, bias):
    x = np.asarray(x, dtype=np.float32)
    edge_index = np.asarray(edge_index)
    self_weight = np.asarray(self_weight, dtype=np.float32)
    neighbor_weight = np.asarray(neighbor_weight, dtype=np.float32)
    bias = np.asarray(bias, dtype=np.float32)

    row = edge_index[0].astype(np.int64)
    col = edge_index[1].astype(np.int64)

    deg = np.bincount(row, minlength=N_NODES).astype(np.float32)
    with np.errstate(divide="ignore"):
        dis = deg ** -0.5

    order = np.argsort(row, kind="stable")
    row_s, col_s = row[order], col[order]
    bounds = np.searchsorted(row_s, np.arange(NC + 1) * ROWS_PER_CORE)

    counts = np.zeros((NC, W, NQ), dtype=np.int64)
    wid = (row_s % ROWS_PER_CORE) // WIN
    qid = col_s // QSIZE
    cid = row_s // ROWS_PER_CORE
    np.add.at(counts, (cid, wid, qid), 1)

    n_chunks, groups, calls, window_chunks, total_tok, total_chunk = (
        _build_schedule(counts))

    nc = _build_program(n_chunks, groups, calls, window_chunks, total_tok,
                        total_chunk)

    # gather table: x pre-scaled by dis[src], + zero sentinel per quarter
    xsc = x * dis[:, None]
    xs = np.zeros((NQ * QROWS, D), dtype=np.float32)
    for q in range(NQ):
        xs[q * QROWS:q * QROWS + QSIZE] = xsc[q * QSIZE:(q + 1) * QSIZE]

    # per-window dis[row] columns, padded to 128 rows on the last window
    disw_full = np.zeros((NC, W * WIN), dtype=np.float32)
    for c in range(NC):
        disw_full[c, :ROWS_PER_CORE] = dis[c * ROWS_PER_CORE:
                                           (c + 1) * ROWS_PER_CORE]
    iota = np.tile(np.arange(128, dtype=np.float32), (128, 1))

    in_maps = []
    for c in range(NC):
        idx_rep, dstl_m = _prep_core(
            c, row_s, col_s, bounds, n_chunks, groups, total_tok, total_chunk)
        in_maps.append({
            "xs": xs,
            "xT": np.ascontiguousarray(
                x[c * ROWS_PER_CORE:(c + 1) * ROWS_PER_CORE].T),
            "idx": idx_rep,
            "dstl": dstl_m,
            "disw": np.ascontiguousarray(
                disw_full[c].reshape(W, WIN).T),
            "ws": self_weight,
            "wn": neighbor_weight,
            "bias": bias.reshape(D, 1),
            "iota": iota,
        })

    global _LAST
    _LAST = (nc, in_maps)
    res = run_bass_kernel_spmd(nc, in_maps, list(range(NC)))
    out = np.empty((N_NODES, D), dtype=np.float32)
    for c in range(NC):
        out[c * ROWS_PER_CORE:(c + 1) * ROWS_PER_CORE] = res.results[c]["outT"].T
    return out


_LAST = None


def profile_exec_ns():
    """Re-run the last-built program with NTFF tracing; returns exec ns."""
    assert _LAST is not None, "call kernel() first"
    nc, in_maps = _LAST
    res = run_bass_kernel_spmd(nc, in_maps, list(range(NC)), trace=True)
    return res.exec_time_ns


# revision 8
# speedup vs baseline: 2.3047x; 1.5429x over previous
"""Trainium2 Bass kernel for BasicGNN message passing.

out = x @ W_s + scatter_add(norm_e * (x @ W_n)[col_e] -> row_e) + bias

Algebraic restructures:
  1. Aggregate x first, transform after (avoids materializing h = x @ W_n):
         agg[r] = sum_{e: row_e=r} norm_e * x[col_e]
         out    = x @ W_s + agg @ W_n + bias
  2. norm is separable: norm_e = dis[row_e] * dis[col_e] with
     dis = deg^-1/2. The dis[col] factor is pre-multiplied into the gather
     table on the host (x' = dis * x); the dis[row] factor is applied once
     per 128-row destination window when copying PSUM -> SBUF.

Sharding: output rows split contiguously across 8 cores; edges partitioned
by destination row. Each core gathers source rows of x' from its own full
copy in DRAM (no collectives).

Device algorithm per core:
  - edges sorted by dst window (128 rows) and source-quarter (dma_gather
    indices are int16, so the table is addressed in 4 quarters of 25001
    rows - the extra row per quarter is a zero sentinel for padding)
  - dma_gather (4 SWDGE queues round-robin) fetches x'[col] rows, 128 per
    chunk (one edge per partition)
  - vector engine builds one-hot S[e, d] = (dst_local[e]==d)
  - tensor engine: psum[d, f] += S.T @ xg accumulates the scatter-add
  - scalar engine applies dis[row] while copying psum -> SBUF
  - per window: transpose agg tile to [f, d] via tensor engine
  - final: out_T = W_s.T @ x_T + W_n.T @ agg_T + bias, streamed per 512 rows
"""

import sys

if "/opt/trn_rl_repo" not in sys.path:
    sys.path.insert(0, "/opt/trn_rl_repo")

import numpy as np

import concourse.bass as bass
import concourse.mybir as mybir
import concourse.tile as tile
from concourse import bacc
from concourse.bass_utils import run_bass_kernel_spmd
from concourse.masks import make_identity

N_NODES = 100000
N_EDGES = 1600000
D = 128
NC = 8
ROWS_PER_CORE = N_NODES // NC          # 12500
WIN = 128                              # dst rows per psum window
W = (ROWS_PER_CORE + WIN - 1) // WIN   # 98 windows per core
NQ = 4                                 # source quarters (int16 index limit)
QSIZE = N_NODES // NQ                  # 25000
QROWS = QSIZE + 1                      # + zero sentinel row per quarter
GROUP = 4                              # windows per gather call group
F32 = mybir.dt.float32
I16 = mybir.dt.int16


def _build_schedule(counts):
    """counts: [NC, W, NQ] edge counts. Returns shared schedule (all cores
    share shapes / loop structure; per-(w,q) chunk counts are max over
    cores)."""
    n_chunks = np.ceil(counts / 128.0).astype(np.int64).max(axis=0)  # [W, NQ]
    groups = [list(range(g, min(g + GROUP, W))) for g in range(0, W, GROUP)]
    calls = []            # (g, q, tok_base, call_chunks, chunk_base)
    window_chunks = {w: [] for w in range(W)}
    tok = 0
    chunk = 0
    for gi, wins in enumerate(groups):
        for q in range(NQ):
            call_chunks = int(sum(n_chunks[w][q] for w in wins))
            if call_chunks == 0:
                continue
            calls.append((gi, q, tok, call_chunks, chunk))
            for w in wins:
                for _ in range(int(n_chunks[w][q])):
                    window_chunks[w].append(chunk)
                    chunk += 1
            tok += call_chunks * 128
    # window-major S/dstl column order: contiguous per window
    wpos = {}
    cnt = 0
    for wins in groups:
        for w in wins:
            for gc in window_chunks[w]:
                wpos[gc] = cnt
                cnt += 1
    assert cnt == chunk
    return n_chunks, groups, calls, window_chunks, wpos, tok, chunk


def _prep_core(core, row_s, col_s, bounds, n_chunks, groups, wpos, total_tok,
               total_chunk):
    """Per-core token arrays: idx (int16 wrapped+replicated) and dstl."""
    lo, hi = bounds[core], bounds[core + 1]
    r = row_s[lo:hi] - core * ROWS_PER_CORE
    c = col_s[lo:hi]
    warr = r // WIN
    qarr = c // QSIZE
    order = np.lexsort((qarr, warr))
    r, c, warr, qarr = r[order], c[order], warr[order], qarr[order]
    dstl = (r % WIN).astype(np.float32)
    cloc = (c % QSIZE).astype(np.int16)

    key = warr * NQ + qarr
    idx_tok = np.full(total_tok, QSIZE, dtype=np.int16)   # sentinel pad
    dstl_tok = np.zeros(total_tok, dtype=np.float32)
    pos = 0
    for wins in groups:
        for q in range(NQ):
            for w in wins:
                k = int(n_chunks[w][q])
                if k == 0:
                    continue
                s = np.searchsorted(key, w * NQ + q, side="left")
                e = np.searchsorted(key, w * NQ + q, side="right")
                m = e - s
                assert m <= k * 128
                idx_tok[pos:pos + m] = cloc[s:e]
                dstl_tok[pos:pos + m] = dstl[s:e]
                pos += k * 128
    assert pos == total_tok

    idx_wrap = idx_tok.reshape(total_tok // 16, 16).T.copy()       # [16, T/16]
    idx_rep = np.tile(idx_wrap, (8, 1))                            # [128, T/16]
    dstl_c = dstl_tok.reshape(total_chunk, 128)                    # [chunk,128]
    dstl_w = np.zeros_like(dstl_c)
    for gc in range(total_chunk):
        dstl_w[wpos[gc]] = dstl_c[gc]
    return idx_rep, dstl_w.T.copy()


def _build_program(n_chunks, groups, calls, window_chunks, wpos, total_tok,
                   total_chunk):
    nc = bacc.Bacc("TRN2", target_bir_lowering=False, debug=False,
                   num_devices=NC, num_swdge_queues=4)
    xs_d = nc.dram_tensor("xs", [NQ * QROWS, D], F32,
                          kind="ExternalInput").ap()
    xT_d = nc.dram_tensor("xT", [D, ROWS_PER_CORE], F32,
                          kind="ExternalInput").ap()
    idx_d = nc.dram_tensor("idx", [128, total_tok // 16], I16,
                           kind="ExternalInput").ap()
    dstl_d = nc.dram_tensor("dstl", [128, total_chunk], F32,
                            kind="ExternalInput").ap()
    disw_d = nc.dram_tensor("disw", [128, W], F32, kind="ExternalInput").ap()
    SB = 8  # chunks per batched S-build
    ws_d = nc.dram_tensor("ws", [D, D], F32, kind="ExternalInput").ap()
    wn_d = nc.dram_tensor("wn", [D, D], F32, kind="ExternalInput").ap()
    bias_d = nc.dram_tensor("bias", [D, 1], F32, kind="ExternalInput").ap()
    iota_d = nc.dram_tensor("iota", [128, 8 * 128], F32,
                           kind="ExternalInput").ap()
    outT_d = nc.dram_tensor("outT", [D, ROWS_PER_CORE], F32,
                            kind="ExternalOutput").ap()

    call_by_group = {}
    for gi, q, tok, cc, cb in calls:
        call_by_group.setdefault(gi, []).append((q, tok, cc, cb))
    group_tok = {gi: (cl[0][1], cl[-1][1] + cl[-1][2] * 128)
                 for gi, cl in call_by_group.items()}

    qn = 0  # SWDGE queue round-robin counter

    with tile.TileContext(nc) as tc:
        with (
            tc.tile_pool(name="const", bufs=1) as constp,
            tc.tile_pool(name="meta", bufs=1) as metap,
            tc.tile_pool(name="aggTp", bufs=1) as aggTp,
            tc.tile_pool(name="idxg", bufs=3) as idxgp,
            tc.tile_pool(name="xg", bufs=8) as xgp,
            tc.tile_pool(name="S", bufs=6) as Sp,
            tc.tile_pool(name="aggsb", bufs=3) as aggsbp,
            tc.tile_pool(name="xT", bufs=2) as xTp,
            tc.tile_pool(name="osb", bufs=2) as osbp,
            tc.tile_pool(name="psA", bufs=2, space="PSUM") as psA,
            tc.tile_pool(name="psT", bufs=2, space="PSUM") as psT,
            tc.tile_pool(name="psO", bufs=2, space="PSUM") as psO,
        ):
            ident = constp.tile([128, 128], F32)
            make_identity(nc, ident[:])
            iota_t = constp.tile([128, 8 * 128], F32)
            nc.sync.dma_start(iota_t[:], iota_d[:])
            ws_t = constp.tile([D, D], F32)
            nc.sync.dma_start(ws_t[:], ws_d[:])
            wn_t = constp.tile([D, D], F32)
            nc.sync.dma_start(wn_t[:], wn_d[:])
            bias_t = constp.tile([D, 1], F32)
            nc.sync.dma_start(bias_t[:], bias_d[:])
            disw_t = constp.tile([128, W], F32)
            nc.sync.dma_start(disw_t[:], disw_d[:])
            dstl_t = metap.tile([128, total_chunk], F32)
            nc.sync.dma_start(dstl_t[:], dstl_d[:])
            aggT = aggTp.tile([128, W * WIN], F32)

            for gi, wins in enumerate(groups):
                t0, t1 = group_tok[gi]
                idx_t = idxgp.tile([128, (t1 - t0) // 16], I16, tag="idxg")
                nc.sync.dma_start(idx_t[:], idx_d[:, t0 // 16:t1 // 16])
                bufs = {}
                for q, tok, cc, cb in call_by_group[gi]:
                    xg = xgp.tile([128, cc * 128], F32, tag="xg")
                    nc.gpsimd.dma_gather(
                        out_ap=xg[:].rearrange("p (c e) -> p c e", e=128),
                        in_ap=xs_d[q * QROWS:(q + 1) * QROWS, :],
                        idxs_ap=idx_t[:, (tok - t0) // 16:
                                      (tok - t0 + cc * 128) // 16],
                        num_idxs=cc * 128,
                        num_idxs_reg=cc * 128,
                        elem_size=D,
                        queue_num=qn % 4,
                        single_packet=False,
                    )
                    qn += 1
                    bufs[q] = (xg, cb)
                for w in wins:
                    chunks = window_chunks[w]
                    psum_agg = psA.tile([128, 128], F32, tag="psA")
                    agg_sb = aggsbp.tile([128, 128], F32, tag="aggsb")
                    if not chunks:
                        nc.vector.memset(agg_sb[:], 0.0)
                    else:
                        # batched one-hot builds, window-major columns
                        wc0 = wpos[chunks[0]]
                        nw = len(chunks)
                        smap = {}
                        for b0 in range(0, nw, SB):
                            k = min(SB, nw - b0)
                            S8 = Sp.tile([128, k * 128], F32, tag="S")
                            nc.vector.tensor_tensor(
                                out=S8[:].rearrange("p (c e) -> p c e", e=128),
                                in0=dstl_t[:, wc0 + b0:wc0 + b0 + k]
                                    .to_broadcast([128, k, 128]),
                                in1=iota_t[:, :k * 128].rearrange(
                                    "p (c e) -> p c e", e=128),
                                op=mybir.AluOpType.is_equal,
                            )
                            for u in range(k):
                                smap[b0 + u] = (S8, u)
                        for ci, gc in enumerate(chunks):
                            q = None
                            for qq, (xgb, cb) in bufs.items():
                                nchq = sum(int(n_chunks[ww][qq])
                                           for ww in wins)
                                if cb <= gc < cb + nchq:
                                    q = qq
                                    break
                            xgb, cb = bufs[q]
                            j = gc - cb
                            S8, u = smap[ci]
                            nc.tensor.matmul(
                                out=psum_agg[:],
                                lhsT=S8[:, u * 128:(u + 1) * 128],
                                rhs=xgb[:, j * 128:(j + 1) * 128],
                                start=(ci == 0),
                                stop=(ci == len(chunks) - 1),
                            )
                        # psum -> sbuf with the dis[row] factor applied
                        nc.scalar.activation(
                            out=agg_sb[:], in_=psum_agg[:],
                            func=mybir.ActivationFunctionType.Copy,
                            scale=disw_t[:, w:w + 1])
                    psum_t = psT.tile([128, 128], F32, tag="psT")
                    nc.tensor.transpose(psum_t[:], agg_sb[:], ident[:])
                    nc.vector.tensor_copy(
                        out=aggT[:, w * WIN:(w + 1) * WIN], in_=psum_t[:])

            # final dense phase: out_T = W_s.T @ x_T + W_n.T @ agg_T + bias
            TS = 512
            for t in range(0, ROWS_PER_CORE, TS):
                n = min(TS, ROWS_PER_CORE - t)
                xT_t = xTp.tile([128, n], F32, tag="xT")
                nc.sync.dma_start(xT_t[:], xT_d[:, t:t + n])
                psum_o = psO.tile([128, n], F32, tag="psO")
                nc.tensor.matmul(out=psum_o[:], lhsT=ws_t[:], rhs=xT_t[:],
                                 start=True, stop=False)
                nc.tensor.matmul(out=psum_o[:], lhsT=wn_t[:],
                                 rhs=aggT[:, t:t + n], start=False, stop=True)
                osb = osbp.tile([128, n], F32, tag="osb")
                nc.vector.tensor_scalar_add(osb[:], psum_o[:], bias_t[:, :1])
                nc.sync.dma_start(outT_d[:, t:t + n], osb[:])
    nc.compile()
    return nc


def kernel(x, edge_index, self_weight, neighbor_weight, bias):
    x = np.asarray(x, dtype=np.float32)
    edge_index = np.asarray(edge_index)
    self_weight = np.asarray(self_weight, dtype=np.float32)
    neighbor_weight = np.asarray(neighbor_weight, dtype=np.float32)
    bias = np.asarray(bias, dtype=np.float32)

    row = edge_index[0].astype(np.int64)
    col = edge_index[1].astype(np.int64)

    deg = np.bincount(row, minlength=N_NODES).astype(np.float32)
    with np.errstate(divide="ignore"):
        dis = deg ** -0.5

    order = np.argsort(row, kind="stable")
    row_s, col_s = row[order], col[order]
    bounds = np.searchsorted(row_s, np.arange(NC + 1) * ROWS_PER_CORE)

    counts = np.zeros((NC, W, NQ), dtype=np.int64)
    wid = (row_s % ROWS_PER_CORE) // WIN
    qid = col_s // QSIZE
    cid = row_s // ROWS_PER_CORE
    np.add.at(counts, (cid, wid, qid), 1)

    n_chunks, groups, calls, window_chunks, wpos, total_tok, total_chunk = (
        _build_schedule(counts))

    nc = _build_program(n_chunks, groups, calls, window_chunks, wpos,
                        total_tok, total_chunk)

    # gather table: x pre-scaled by dis[src], + zero sentinel per quarter
    xsc = x * dis[:, None]
    xs = np.zeros((NQ * QROWS, D), dtype=np.float32)
    for q in range(NQ):
        xs[q * QROWS:q * QROWS + QSIZE] = xsc[q * QSIZE:(q + 1) * QSIZE]

    # per-window dis[row] columns, padded to 128 rows on the last window
    disw_full = np.zeros((NC, W * WIN), dtype=np.float32)
    for c in range(NC):
        disw_full[c, :ROWS_PER_CORE] = dis[c * ROWS_PER_CORE:
                                           (c + 1) * ROWS_PER_CORE]
    iota = np.tile(np.arange(128, dtype=np.float32), (128, 8))

    in_maps = []
    for c in range(NC):
        idx_rep, dstl_m = _prep_core(
            c, row_s, col_s, bounds, n_chunks, groups, wpos, total_tok,
            total_chunk)
        in_maps.append({
            "xs": xs,
            "xT": np.ascontiguousarray(
                x[c * ROWS_PER_CORE:(c + 1) * ROWS_PER_CORE].T),
            "idx": idx_rep,
            "dstl": dstl_m,
            "disw": np.ascontiguousarray(
                disw_full[c].reshape(W, WIN).T),
            "ws": self_weight,
            "wn": neighbor_weight,
            "bias": bias.reshape(D, 1),
            "iota": iota,
        })

    global _LAST
    _LAST = (nc, in_maps)
    res = run_bass_kernel_spmd(nc, in_maps, list(range(NC)))
    out = np.empty((N_NODES, D), dtype=np.float32)
    for c in range(NC):
        out[c * ROWS_PER_CORE:(c + 1) * ROWS_PER_CORE] = res.results[c]["outT"].T
    return out


_LAST = None


def profile_exec_ns():
    """Re-run the last-built program with NTFF tracing; returns exec ns."""
    assert _LAST is not None, "call kernel() first"
    nc, in_maps = _LAST
    res = run_bass_kernel_spmd(nc, in_maps, list(range(NC)), trace=True)
    return res.exec_time_ns


# revision 9
# speedup vs baseline: 2.7181x; 1.1794x over previous
"""Trainium2 Bass kernel for BasicGNN message passing.

out = x @ W_s + scatter_add(norm_e * (x @ W_n)[col_e] -> row_e) + bias

Algebraic restructures:
  1. Aggregate x first, transform after (avoids materializing h = x @ W_n):
         agg[r] = sum_{e: row_e=r} norm_e * x[col_e]
         out    = x @ W_s + agg @ W_n + bias
  2. norm is separable: norm_e = dis[row_e] * dis[col_e] with
     dis = deg^-1/2. The dis[col] factor is pre-multiplied into the gather
     table on the host (x' = dis * x); the dis[row] factor is applied once
     per 128-row destination window when copying PSUM -> SBUF.

Sharding: output rows split contiguously across 8 cores; edges partitioned
by destination row. Each core gathers source rows of x' from its own full
copy in DRAM (no collectives).

Device algorithm per core:
  - edges sorted by dst window (128 rows) and source-quarter (dma_gather
    indices are int16, so the table is addressed in 4 quarters of 25001
    rows - the extra row per quarter is a zero sentinel for padding)
  - dma_gather (4 SWDGE queues round-robin) fetches x'[col] rows, 128 per
    chunk (one edge per partition)
  - vector engine builds one-hot S[e, d] = (dst_local[e]==d)
  - tensor engine: psum[d, f] += S.T @ xg accumulates the scatter-add
  - scalar engine applies dis[row] while copying psum -> SBUF
  - per window: transpose agg tile to [f, d] via tensor engine
  - final: out_T = W_s.T @ x_T + W_n.T @ agg_T + bias, streamed per 512 rows
"""

import sys

if "/opt/trn_rl_repo" not in sys.path:
    sys.path.insert(0, "/opt/trn_rl_repo")

import numpy as np

import concourse.bass as bass
import concourse.mybir as mybir
import concourse.tile as tile
from concourse import bacc
from concourse.bass_utils import run_bass_kernel_spmd
from concourse.masks import make_identity

N_NODES = 100000
N_EDGES = 1600000
D = 128
NC = 8
ROWS_PER_CORE = N_NODES // NC          # 12500
WIN = 128                              # dst rows per psum window
W = (ROWS_PER_CORE + WIN - 1) // WIN   # 98 windows per core
NQ = 4                                 # source quarters (int16 index limit)
QSIZE = N_NODES // NQ                  # 25000
QROWS = QSIZE + 1                      # + zero sentinel row per quarter
GROUP = 4                              # windows per gather call group
F32 = mybir.dt.float32
BF16 = mybir.dt.bfloat16
I16 = mybir.dt.int16
USE_BF16 = True            # bf16 gather table + S matrices (psum stays fp32)
EDT = BF16 if USE_BF16 else F32


def _build_schedule(counts):
    """counts: [NC, W, NQ] edge counts. Returns shared schedule (all cores
    share shapes / loop structure; per-(w,q) chunk counts are max over
    cores)."""
    n_chunks = np.ceil(counts / 128.0).astype(np.int64).max(axis=0)  # [W, NQ]
    groups = [list(range(g, min(g + GROUP, W))) for g in range(0, W, GROUP)]
    calls = []            # (g, q, tok_base, call_chunks, chunk_base)
    window_chunks = {w: [] for w in range(W)}
    tok = 0
    chunk = 0
    for gi, wins in enumerate(groups):
        for q in range(NQ):
            call_chunks = int(sum(n_chunks[w][q] for w in wins))
            if call_chunks == 0:
                continue
            calls.append((gi, q, tok, call_chunks, chunk))
            for w in wins:
                for _ in range(int(n_chunks[w][q])):
                    window_chunks[w].append(chunk)
                    chunk += 1
            tok += call_chunks * 128
    # window-major S/dstl column order: contiguous per window
    wpos = {}
    cnt = 0
    for wins in groups:
        for w in wins:
            for gc in window_chunks[w]:
                wpos[gc] = cnt
                cnt += 1
    assert cnt == chunk
    return n_chunks, groups, calls, window_chunks, wpos, tok, chunk


def _prep_core(core, row_s, col_s, bounds, n_chunks, groups, wpos, total_tok,
               total_chunk):
    """Per-core token arrays: idx (int16 wrapped+replicated) and dstl."""
    lo, hi = bounds[core], bounds[core + 1]
    r = row_s[lo:hi] - core * ROWS_PER_CORE
    c = col_s[lo:hi]
    warr = r // WIN
    qarr = c // QSIZE
    order = np.lexsort((qarr, warr))
    r, c, warr, qarr = r[order], c[order], warr[order], qarr[order]
    dstl = (r % WIN).astype(np.float32)
    cloc = (c % QSIZE).astype(np.int16)

    key = warr * NQ + qarr
    idx_tok = np.full(total_tok, QSIZE, dtype=np.int16)   # sentinel pad
    dstl_tok = np.zeros(total_tok, dtype=np.float32)
    pos = 0
    for wins in groups:
        for q in range(NQ):
            for w in wins:
                k = int(n_chunks[w][q])
                if k == 0:
                    continue
                s = np.searchsorted(key, w * NQ + q, side="left")
                e = np.searchsorted(key, w * NQ + q, side="right")
                m = e - s
                assert m <= k * 128
                idx_tok[pos:pos + m] = cloc[s:e]
                dstl_tok[pos:pos + m] = dstl[s:e]
                pos += k * 128
    assert pos == total_tok

    idx_wrap = idx_tok.reshape(total_tok // 16, 16).T.copy()       # [16, T/16]
    idx_rep = np.tile(idx_wrap, (8, 1))                            # [128, T/16]
    dstl_c = dstl_tok.reshape(total_chunk, 128)                    # [chunk,128]
    dstl_w = np.zeros_like(dstl_c)
    for gc in range(total_chunk):
        dstl_w[wpos[gc]] = dstl_c[gc]
    return idx_rep, dstl_w.T.copy()


def _build_program(n_chunks, groups, calls, window_chunks, wpos, total_tok,
                   total_chunk):
    nc = bacc.Bacc("TRN2", target_bir_lowering=False, debug=False,
                   num_devices=NC, num_swdge_queues=4)
    xs_d = nc.dram_tensor("xs", [NQ * QROWS, D], EDT,
                          kind="ExternalInput").ap()
    xT_d = nc.dram_tensor("xT", [D, ROWS_PER_CORE], F32,
                          kind="ExternalInput").ap()
    idx_d = nc.dram_tensor("idx", [128, total_tok // 16], I16,
                           kind="ExternalInput").ap()
    dstl_d = nc.dram_tensor("dstl", [128, total_chunk], EDT,
                            kind="ExternalInput").ap()
    disw_d = nc.dram_tensor("disw", [128, W], F32, kind="ExternalInput").ap()
    SB = 8  # chunks per batched S-build
    ws_d = nc.dram_tensor("ws", [D, D], F32, kind="ExternalInput").ap()
    wn_d = nc.dram_tensor("wn", [D, D], F32, kind="ExternalInput").ap()
    bias_d = nc.dram_tensor("bias", [D, 1], F32, kind="ExternalInput").ap()
    iota_d = nc.dram_tensor("iota", [128, 8 * 128], EDT,
                           kind="ExternalInput").ap()
    outT_d = nc.dram_tensor("outT", [D, ROWS_PER_CORE], F32,
                            kind="ExternalOutput").ap()

    call_by_group = {}
    for gi, q, tok, cc, cb in calls:
        call_by_group.setdefault(gi, []).append((q, tok, cc, cb))
    group_tok = {gi: (cl[0][1], cl[-1][1] + cl[-1][2] * 128)
                 for gi, cl in call_by_group.items()}

    qn = 0  # SWDGE queue round-robin counter

    with tile.TileContext(nc) as tc:
        with (
            tc.tile_pool(name="const", bufs=1) as constp,
            tc.tile_pool(name="meta", bufs=1) as metap,
            tc.tile_pool(name="aggTp", bufs=1) as aggTp,
            tc.tile_pool(name="idxg", bufs=3) as idxgp,
            tc.tile_pool(name="xg", bufs=8) as xgp,
            tc.tile_pool(name="S", bufs=6) as Sp,
            tc.tile_pool(name="aggsb", bufs=3) as aggsbp,
            tc.tile_pool(name="xT", bufs=2) as xTp,
            tc.tile_pool(name="osb", bufs=2) as osbp,
            tc.tile_pool(name="psA", bufs=2, space="PSUM") as psA,
            tc.tile_pool(name="psT", bufs=2, space="PSUM") as psT,
            tc.tile_pool(name="psO", bufs=2, space="PSUM") as psO,
        ):
            ident = constp.tile([128, 128], F32)
            make_identity(nc, ident[:])
            iota_t = constp.tile([128, 8 * 128], EDT)
            nc.sync.dma_start(iota_t[:], iota_d[:])
            ws_t = constp.tile([D, D], F32)
            nc.sync.dma_start(ws_t[:], ws_d[:])
            wn_t = constp.tile([D, D], F32)
            nc.sync.dma_start(wn_t[:], wn_d[:])
            bias_t = constp.tile([D, 1], F32)
            nc.sync.dma_start(bias_t[:], bias_d[:])
            disw_t = constp.tile([128, W], F32)
            nc.sync.dma_start(disw_t[:], disw_d[:])
            dstl_t = metap.tile([128, total_chunk], EDT)
            nc.sync.dma_start(dstl_t[:], dstl_d[:])
            aggT = aggTp.tile([128, W * WIN], F32)

            for gi, wins in enumerate(groups):
                t0, t1 = group_tok[gi]
                idx_t = idxgp.tile([128, (t1 - t0) // 16], I16, tag="idxg")
                nc.sync.dma_start(idx_t[:], idx_d[:, t0 // 16:t1 // 16])
                bufs = {}
                for q, tok, cc, cb in call_by_group[gi]:
                    xg = xgp.tile([128, cc * 128], EDT, tag="xg")
                    nc.gpsimd.dma_gather(
                        out_ap=xg[:].rearrange("p (c e) -> p c e", e=128),
                        in_ap=xs_d[q * QROWS:(q + 1) * QROWS, :],
                        idxs_ap=idx_t[:, (tok - t0) // 16:
                                      (tok - t0 + cc * 128) // 16],
                        num_idxs=cc * 128,
                        num_idxs_reg=cc * 128,
                        elem_size=D,
                        queue_num=qn % 4,
                        single_packet=False,
                    )
                    qn += 1
                    bufs[q] = (xg, cb)
                for w in wins:
                    chunks = window_chunks[w]
                    psum_agg = psA.tile([128, 128], F32, tag="psA")
                    agg_sb = aggsbp.tile([128, 128], F32, tag="aggsb")
                    if not chunks:
                        nc.vector.memset(agg_sb[:], 0.0)
                    else:
                        # batched one-hot builds, window-major columns
                        wc0 = wpos[chunks[0]]
                        nw = len(chunks)
                        smap = {}
                        for b0 in range(0, nw, SB):
                            k = min(SB, nw - b0)
                            S8 = Sp.tile([128, k * 128], EDT, tag="S")
                            nc.vector.tensor_tensor(
                                out=S8[:].rearrange("p (c e) -> p c e", e=128),
                                in0=dstl_t[:, wc0 + b0:wc0 + b0 + k]
                                    .to_broadcast([128, k, 128]),
                                in1=iota_t[:, :k * 128].rearrange(
                                    "p (c e) -> p c e", e=128),
                                op=mybir.AluOpType.is_equal,
                            )
                            for u in range(k):
                                smap[b0 + u] = (S8, u)
                        for ci, gc in enumerate(chunks):
                            q = None
                            for qq, (xgb, cb) in bufs.items():
                                nchq = sum(int(n_chunks[ww][qq])
                                           for ww in wins)
                                if cb <= gc < cb + nchq:
                                    q = qq
                                    break
                            xgb, cb = bufs[q]
                            j = gc - cb
                            S8, u = smap[ci]
                            nc.tensor.matmul(
                                out=psum_agg[:],
                                lhsT=S8[:, u * 128:(u + 1) * 128],
                                rhs=xgb[:, j * 128:(j + 1) * 128],
                                start=(ci == 0),
                                stop=(ci == len(chunks) - 1),
                            )
                        # psum -> sbuf with the dis[row] factor applied
                        nc.scalar.activation(
                            out=agg_sb[:], in_=psum_agg[:],
                            func=mybir.ActivationFunctionType.Copy,
                            scale=disw_t[:, w:w + 1])
                    psum_t = psT.tile([128, 128], F32, tag="psT")
                    nc.tensor.transpose(psum_t[:], agg_sb[:], ident[:])
                    nc.vector.tensor_copy(
                        out=aggT[:, w * WIN:(w + 1) * WIN], in_=psum_t[:])

            # final dense phase: out_T = W_s.T @ x_T + W_n.T @ agg_T + bias
            TS = 512
            for t in range(0, ROWS_PER_CORE, TS):
                n = min(TS, ROWS_PER_CORE - t)
                xT_t = xTp.tile([128, n], F32, tag="xT")
                nc.sync.dma_start(xT_t[:], xT_d[:, t:t + n])
                psum_o = psO.tile([128, n], F32, tag="psO")
                nc.tensor.matmul(out=psum_o[:], lhsT=ws_t[:], rhs=xT_t[:],
                                 start=True, stop=False)
                nc.tensor.matmul(out=psum_o[:], lhsT=wn_t[:],
                                 rhs=aggT[:, t:t + n], start=False, stop=True)
                osb = osbp.tile([128, n], F32, tag="osb")
                nc.vector.tensor_scalar_add(osb[:], psum_o[:], bias_t[:, :1])
                nc.sync.dma_start(outT_d[:, t:t + n], osb[:])
    nc.compile()
    return nc


def kernel(x, edge_index, self_weight, neighbor_weight, bias):
    x = np.asarray(x, dtype=np.float32)
    edge_index = np.asarray(edge_index)
    self_weight = np.asarray(self_weight, dtype=np.float32)
    neighbor_weight = np.asarray(neighbor_weight, dtype=np.float32)
    bias = np.asarray(bias, dtype=np.float32)

    row = edge_index[0].astype(np.int64)
    col = edge_index[1].astype(np.int64)

    deg = np.bincount(row, minlength=N_NODES).astype(np.float32)
    with np.errstate(divide="ignore"):
        dis = deg ** -0.5

    order = np.argsort(row, kind="stable")
    row_s, col_s = row[order], col[order]
    bounds = np.searchsorted(row_s, np.arange(NC + 1) * ROWS_PER_CORE)

    counts = np.zeros((NC, W, NQ), dtype=np.int64)
    wid = (row_s % ROWS_PER_CORE) // WIN
    qid = col_s // QSIZE
    cid = row_s // ROWS_PER_CORE
    np.add.at(counts, (cid, wid, qid), 1)

    n_chunks, groups, calls, window_chunks, wpos, total_tok, total_chunk = (
        _build_schedule(counts))

    nc = _build_program(n_chunks, groups, calls, window_chunks, wpos,
                        total_tok, total_chunk)

    # gather table: x pre-scaled by dis[src], + zero sentinel per quarter
    import ml_dtypes
    edt_np = ml_dtypes.bfloat16 if USE_BF16 else np.float32
    xsc = x * dis[:, None]
    xs = np.zeros((NQ * QROWS, D), dtype=edt_np)
    for q in range(NQ):
        xs[q * QROWS:q * QROWS + QSIZE] = xsc[q * QSIZE:(q + 1) * QSIZE]

    # per-window dis[row] columns, padded to 128 rows on the last window
    disw_full = np.zeros((NC, W * WIN), dtype=np.float32)
    for c in range(NC):
        disw_full[c, :ROWS_PER_CORE] = dis[c * ROWS_PER_CORE:
                                           (c + 1) * ROWS_PER_CORE]
    iota = np.tile(np.arange(128, dtype=np.float32), (128, 8)).astype(edt_np)

    in_maps = []
    for c in range(NC):
        idx_rep, dstl_m = _prep_core(
            c, row_s, col_s, bounds, n_chunks, groups, wpos, total_tok,
            total_chunk)
        in_maps.append({
            "xs": xs,
            "xT": np.ascontiguousarray(
                x[c * ROWS_PER_CORE:(c + 1) * ROWS_PER_CORE].T),
            "idx": idx_rep,
            "dstl": dstl_m.astype(edt_np),
            "disw": np.ascontiguousarray(
                disw_full[c].reshape(W, WIN).T),
            "ws": self_weight,
            "wn": neighbor_weight,
            "bias": bias.reshape(D, 1),
            "iota": iota,
        })

    global _LAST
    _LAST = (nc, in_maps)
    res = run_bass_kernel_spmd(nc, in_maps, list(range(NC)))
    out = np.empty((N_NODES, D), dtype=np.float32)
    for c in range(NC):
        out[c * ROWS_PER_CORE:(c + 1) * ROWS_PER_CORE] = res.results[c]["outT"].T
    return out


_LAST = None


def profile_exec_ns():
    """Re-run the last-built program with NTFF tracing; returns exec ns."""
    assert _LAST is not None, "call kernel() first"
    nc, in_maps = _LAST
    res = run_bass_kernel_spmd(nc, in_maps, list(range(NC)), trace=True)
    return res.exec_time_ns
